# revision 1
# baseline (speedup 1.0000x reference)
"""Cross multi-head attention + residual + LayerNorm on 8 Trainium2 NeuronCores.

Reference (per batch b):
    q = x_q @ Wq.T + bq ; k = x_kv @ Wk.T + bk ; v = x_kv @ Wv.T + bv
    per head: ctx = softmax(q k^T / sqrt(64)) v
    out = concat(ctx) @ Wo.T + bo ;  y = LayerNorm(out + x_q) * gamma + beta

Sharding (8 cores): data parallel on batch (2 groups of 4 cores), tensor
parallel on heads (4 of 16 heads per core). Each core computes q/k/v
projections for its 4 heads over the full sequences, attention, and a
partial output projection (its heads' slice of Wo columns); a ReduceScatter
within each 4-core group sums the partials and hands each core 1/4 of the
rows, on which it applies residual + LayerNorm locally (bo is pre-added
into the residual host-side).

Compute path is bf16 (inputs/weights converted host-side), with fp32 PSUM
accumulation, fp32 softmax denominators, and fp32 residual + LayerNorm.
x^T needed by the projections is produced by DMA x-bar transpose loads
(bf16-only HW path) instead of PE transposes.  Softmax skips
max-subtraction (scores ~ N(0,1)) and folds the 1/8 scale into the ACT
exp; the denominator comes from an all-ones column appended to each head's
V block, so the context matmul yields [ctx; denom] in one PSUM pass.

Self-contained: hardcodes shapes for B=2, L=2048, E=1024, H=16, Dh=64.
"""

from contextlib import ExitStack

import numpy as np
import ml_dtypes

import concourse.bass as bass
import concourse.mybir as mybir
import concourse.tile as tile
from concourse.bass_test_utils import run_kernel

F32 = mybir.dt.float32
BF16 = mybir.dt.bfloat16
NP_BF16 = ml_dtypes.bfloat16

B = 2
L = 2048          # query and kv sequence length
E = 1024          # embed
H_LOC = 4         # heads per core
DH = 64
EC = E // 128     # 8 e-chunks
JC = L // 128     # 16 sequence chunks of 128
IT = 512          # i-tile (moving free dim) for scores/ctx
N_IT = L // IT    # 4
GROUPS = [[0, 1, 2, 3], [4, 5, 6, 7]]
LN_EPS = 1e-5


def make_attention_kernel(iters=1, tail=True):
    def _k(tc, outs, ins):
        return _attention_body(tc, outs, ins, iters, tail)
    return _k


def _attention_body(tc: tile.TileContext, outs, ins, iters, tail=True):
    nc = tc.nc
    (out,) = outs            # [4, 128, 1024] f32 four row-bands of the final output
    (xq, xkv, wqT, wkT, wvT, woT, bqk, bv, gamma, beta, xqr) = ins
    # xq/xkv: [2048, 1024] bf16 (full batch seqs)
    # wqT/wkT/wvT: [1024, 256] bf16 (W.T slice for this core's 4 heads)
    # woT: [256, 1024] bf16 (Wo cols slice, transposed)
    # bqk: [128, 4] f32 (cols: bq pair0, bq pair1, bk pair0, bk pair1)
    # bv: [256] f32 ; gamma/beta: [1024] f32
    # xqr: [4, 128, 1024] f32 residual rows (+bo) matching this core's RS rows

    rs_in = [nc.dram_tensor(f"rs_in{k}", [IT, E], BF16) for k in range(4)]
    rs_out = [nc.dram_tensor(f"rs_out{k}", [128, E], BF16) for k in range(4)]

    ctx = ExitStack()
    singles = ctx.enter_context(tc.tile_pool(name="singles", bufs=1))
    big = ctx.enter_context(tc.tile_pool(name="big", bufs=1))
    ex_pool = ctx.enter_context(tc.tile_pool(name="ex", bufs=6))
    norm = ctx.enter_context(tc.tile_pool(name="norm", bufs=3))
    small = ctx.enter_context(tc.tile_pool(name="small", bufs=2))
    evac = ctx.enter_context(tc.tile_pool(name="evac", bufs=2))
    ps = ctx.enter_context(tc.tile_pool(name="ps", bufs=2, space="PSUM"))
    psc = ctx.enter_context(tc.tile_pool(name="psc", bufs=2, space="PSUM"))

    # ---- constants / weights -------------------------------------------------
    w_sb = {}
    for name, src, shape in (
        ("wq", wqT, [128, EC, 256]),
        ("wk", wkT, [128, EC, 256]),
        ("wv", wvT, [128, EC, 256]),
        ("wo", woT, [128, 2, E]),
    ):
        wr = singles.tile(shape, BF16, name=f"{name}_r")
        nc.sync.dma_start(out=wr[:], in_=src.rearrange("(c p) n -> p c n", p=128))
        w_sb[name] = wr

    bqk_sb = singles.tile([128, 4], F32, name="bqk_sb")
    nc.sync.dma_start(out=bqk_sb[:], in_=bqk[:])
    bv_bc = singles.tile([128, 256], F32, name="bv_bc")
    nc.gpsimd.dma_start(out=bv_bc[:], in_=bv[None, :].to_broadcast([128, 256]))
    gamma_bc = singles.tile([128, E], F32, name="gamma_bc")
    nc.gpsimd.dma_start(out=gamma_bc[:], in_=gamma[None, :].to_broadcast([128, E]))
    beta_bc = singles.tile([128, E], F32, name="beta_bc")
    nc.gpsimd.dma_start(out=beta_bc[:], in_=beta[None, :].to_broadcast([128, E]))
    eps_sb = singles.tile([128, 1], F32, name="eps_sb")
    nc.vector.memset(eps_sb[:], LN_EPS)


    # v' tile: [128 part(j%128), 16 (j//128), 4*65] ; col 64 of each head
    # block is the all-ones denominator column.
    v_sb = big.tile([128, JC, H_LOC * 65], BF16, name="v_sb")
    ones_sb = singles.tile([128, JC], F32, name="ones_sb")
    nc.vector.memset(ones_sb[:], 1.0)
    for h in range(H_LOC):
        nc.vector.tensor_copy(
            v_sb[:, :, h * 65 + 64 : h * 65 + 65], ones_sb[:, :, None]
        )

    xkvT = big.tile([128, EC, L], BF16, name="xkvT")   # [e%128, e//128, j]
    xqT = big.tile([128, EC, L], BF16, name="xqT")     # [e%128, e//128, i]
    kT_sb = big.tile([128, 2, L], BF16, name="kT_sb")  # [d(pair), pair, j]
    qT_sb = big.tile([128, 2, L], BF16, name="qT_sb")  # [d(pair), pair, i]
    ctxT_sb = big.tile([128, 2, L], BF16, name="ctxT_sb")  # [hd%128, hd//2? see use]

    def load_T(src, dst, tl):
        for ec in range(EC):
            nc.sync.dma_start(
                out=dst[:, ec, tl * IT : (tl + 1) * IT],
                in_=src[tl * IT : (tl + 1) * IT, ec * 128 : (ec + 1) * 128],
                transpose=True,
            )

    def kv_proj(jt):
        for pair in range(2):
            pk = ps.tile([128, IT], F32, name=f"pk_{jt}_{pair}", tag="ps_s")
            for ec in range(EC):
                nc.tensor.matmul(
                    pk[:],
                    w_sb["wk"][:, ec, pair * 128 : (pair + 1) * 128],
                    xkvT[:, ec, jt * IT : (jt + 1) * IT],
                    start=(ec == 0),
                    stop=(ec == EC - 1),
                )
            nc.vector.tensor_scalar(
                out=kT_sb[:, pair, jt * IT : (jt + 1) * IT],
                in0=pk[:],
                scalar1=bqk_sb[:, 2 + pair : 3 + pair],
                scalar2=None,
                op0=mybir.AluOpType.add,
            )
        for jj in range(4):
            jc = jt * 4 + jj
            pv = ps.tile([128, 256], F32, name=f"pv_{jc}", tag="ps_s")
            for ec in range(EC):
                nc.tensor.matmul(
                    pv[:],
                    xkvT[:, ec, jc * 128 : (jc + 1) * 128],
                    w_sb["wv"][:, ec, :],
                    start=(ec == 0),
                    stop=(ec == EC - 1),
                )
            nc.vector.tensor_tensor(
                out=v_sb[:, jc, :].rearrange("p (h d) -> p h d", d=65)[:, :, 0:64],
                in0=pv.rearrange("p (h d) -> p h d", d=64),
                in1=bv_bc.rearrange("p (h d) -> p h d", d=64),
                op=mybir.AluOpType.add,
            )

    def q_proj(it):
        for pair in range(2):
            pq = ps.tile([128, IT], F32, name=f"pq_{it}_{pair}", tag="ps_s")
            for ec in range(EC):
                nc.tensor.matmul(
                    pq[:],
                    w_sb["wq"][:, ec, pair * 128 : (pair + 1) * 128],
                    xqT[:, ec, it * IT : (it + 1) * IT],
                    start=(ec == 0),
                    stop=(ec == EC - 1),
                )
            nc.vector.tensor_scalar(
                out=qT_sb[:, pair, it * IT : (it + 1) * IT],
                in0=pq[:],
                scalar1=bqk_sb[:, pair : pair + 1],
                scalar2=None,
                op0=mybir.AluOpType.add,
            )

    def attn_chunk(pair, it, pc_a, pc_b, jcs):
        ha, hb = 2 * pair, 2 * pair + 1
        for jc in jcs:
            s_ps = ps.tile([128, 2, IT], F32, name=f"sps_{pair}_{it}_{jc}",
                           tag="ps_s2")
            nc.tensor.matmul(
                s_ps[:, 0, :],
                kT_sb[0:64, pair, jc * 128 : (jc + 1) * 128],
                qT_sb[0:64, pair, it * IT : (it + 1) * IT],
                start=True,
                stop=True,
                tile_position=(0, 0),
            )
            nc.tensor.matmul(
                s_ps[:, 1, :],
                kT_sb[64:128, pair, jc * 128 : (jc + 1) * 128],
                qT_sb[64:128, pair, it * IT : (it + 1) * IT],
                start=True,
                stop=True,
                tile_position=(64, 0),
            )
            ex = ex_pool.tile([128, 2, IT], BF16, name=f"ex_{pair}_{it}_{jc}",
                              tag="ex")
            nc.scalar.activation(
                out=ex[:],
                in_=s_ps[:],
                func=mybir.ActivationFunctionType.Exp,
                scale=0.125,
            )
            nc.tensor.matmul(
                pc_a[0:65, :],
                v_sb[:, jc, ha * 65 : (ha + 1) * 65],
                ex[:, 0, :],
                start=(jc == 0),
                stop=(jc == JC - 1),
            )
            nc.tensor.matmul(
                pc_b[0:65, :],
                v_sb[:, jc, hb * 65 : (hb + 1) * 65],
                ex[:, 1, :],
                start=(jc == 0),
                stop=(jc == JC - 1),
            )

    def attn_norm(pair, it, pc_a, pc_b):
        # Evacuate the PSUM accumulators to SBUF immediately so the psc ring
        # frees for the next slot.  rows 0-63 are ctx^T, row 64 the softmax
        # denominator; reciprocal in place, then a PE matmul with sel64
        # broadcasts the reciprocal row to 64 partitions on-chip (no DRAM
        # round trip), and DVE applies it.
        pcs = norm.tile([128, 2, IT], mybir.dt.float32r,
                        name=f"pcs_{pair}_{it}", tag="pcs")
        nc.vector.tensor_copy(pcs[:, 0, :], pc_a[:])
        nc.vector.tensor_copy(pcs[:, 1, :], pc_b[:])
        with nc.allow_low_precision(reason="f32r has identical fp32 storage"):
            nc.vector.reciprocal(pcs[64:65, 0, :], pcs[64:65, 0, :])
            nc.vector.reciprocal(pcs[64:65, 1, :], pcs[64:65, 1, :])
        # broadcast the reciprocal rows across partitions on the nearly idle
        # GpSimd engine: stage partition 64 -> 0 with a small SBUF->SBUF
        # copy, then partition_broadcast (which reads physical partition 0).
        # No DRAM round trip, no PE/DVE queue involvement.
        stg = norm.tile([1, 2, IT], mybir.dt.float32r,
                        name=f"stg_{pair}_{it}", tag="stg")
        nc.gpsimd.dma_start(out=stg[:], in_=pcs[64:65, :, :])
        bc = norm.tile([64, 2, IT], mybir.dt.float32r,
                       name=f"bc_{pair}_{it}", tag="bc")
        nc.gpsimd.partition_broadcast(bc[:, 0, :], stg[0:1, 0, :], channels=64)
        nc.gpsimd.partition_broadcast(bc[:, 1, :], stg[0:1, 1, :], channels=64)
        return pcs, bc

    def attn_finish(pair, it, pcs, bc):
        # Deferred one slot so the DVE queue never head-of-line blocks on the
        # broadcast chain.
        nc.vector.tensor_tensor(
            out=ctxT_sb[0:64, pair, it * IT : (it + 1) * IT],
            in0=pcs[0:64, 0, :],
            in1=bc[:, 0, :],
            op=mybir.AluOpType.mult,
        )
        sc = norm.tile([64, IT], BF16, name=f"sc_{pair}_{it}", tag="sc")
        nc.vector.tensor_tensor(
            out=sc[:],
            in0=pcs[0:64, 1, :],
            in1=bc[:, 1, :],
            op=mybir.AluOpType.mult,
        )
        nc.gpsimd.dma_start(
            out=ctxT_sb[64:128, pair, it * IT : (it + 1) * IT],
            in_=sc[:],
        )

    def attn_slot(pair, it):
        pc_a = psc.tile([128, IT], F32, name=f"pca_{pair}_{it}", tag="ps_c")
        pc_b = psc.tile([128, IT], F32, name=f"pcb_{pair}_{it}", tag="ps_c")
        attn_chunk(pair, it, pc_a, pc_b, range(JC))
        return attn_norm(pair, it, pc_a, pc_b)

    def outproj_band(it, do_tail):
        # po tiles use the projection PSUM ring (free during attention) so the
        # scores ring (ps_s2) never stalls on output-projection evacuation.
        # The whole band is staged in one SBUF tile and shipped with a single
        # 1 MB DMA (4 small DMAs would pay 4x the fixed cost).
        ot = evac.tile([128, 4, E], BF16, name=f"ot_{it}", tag="ot")
        for i2 in range(4):
            ic = it * 4 + i2
            for et in range(2):
                po = ps.tile([128, IT], F32, name=f"po_{ic}_{et}", tag="ps_s")
                for hc in range(2):
                    nc.tensor.matmul(
                        po[:],
                        ctxT_sb[:, hc, ic * 128 : (ic + 1) * 128],
                        w_sb["wo"][:, hc, et * IT : (et + 1) * IT],
                        start=(hc == 0),
                        stop=(hc == 1),
                    )
                nc.vector.tensor_copy(ot[:, i2, et * IT : (et + 1) * IT], po[:])
        nc.sync.dma_start(
            out=rs_in[it].ap().rearrange("(a p) n -> p a n", p=128), in_=ot[:]
        )
        if do_tail:
            nc.gpsimd.collective_compute(
                "ReduceScatter",
                mybir.AluOpType.add,
                replica_groups=GROUPS,
                ins=[rs_in[it].ap()],
                outs=[rs_out[it].ap()],
            )

    def load_all(kv_first=True):
        load_T(xkv, xkvT, 0)
        load_T(xq, xqT, 0)
        for tl in (1, 2, 3):
            load_T(xkv, xkvT, tl)
        for tl in (1, 2, 3):
            load_T(xq, xqT, tl)

    def body(do_tail=True, lead_load=True, prefetch=False):
        # x^T DMA x-bar transpose loads (skipped when the previous loop
        # iteration already prefetched them)
        if lead_load:
            load_all()

        # slot (0,0) starts as soon as kv tile 0 is projected; remaining kv
        # projections interleave with its attention chunks so ACT fills early.
        kv_proj(0)
        q_proj(0)
        pc_a0 = psc.tile([128, IT], F32, name="pca_0_0", tag="ps_c")
        pc_b0 = psc.tile([128, IT], F32, name="pcb_0_0", tag="ps_c")
        attn_chunk(0, 0, pc_a0, pc_b0, range(0, 4))
        kv_proj(1)
        attn_chunk(0, 0, pc_a0, pc_b0, range(4, 8))
        kv_proj(2)
        attn_chunk(0, 0, pc_a0, pc_b0, range(8, 12))
        kv_proj(3)
        attn_chunk(0, 0, pc_a0, pc_b0, range(12, 16))
        prev = (0, 0) + attn_norm(0, 0, pc_a0, pc_b0)

        # Each slot's normalize-finish (and each band's outproj) is emitted
        # one slot late so no engine queue head-of-line blocks on the
        # normalize chain; band it's outproj lands after band it+1 starts.
        for it in range(N_IT):
            for pair in range(2):
                if (pair, it) == (0, 0):
                    continue
                if pair == 0:
                    q_proj(it)
                cur = (pair, it) + attn_slot(pair, it)
                attn_finish(*prev)
                if pair == 1 and it > 0:
                    outproj_band(it - 1, do_tail)
                prev = cur
        if prefetch:
            load_all()
        attn_finish(*prev)
        outproj_band(N_IT - 1, do_tail)

        # ---- residual + LayerNorm per received band -------------------------
        for band in (range(4) if do_tail else []):
            xt = evac.tile([128, E], F32, name=f"xt_{band}", tag="xt")
            nc.gpsimd.dma_start(out=xt[:], in_=rs_out[band].ap())  # bf16 -> f32 cast
            xr = evac.tile([128, E], F32, name=f"xr_{band}", tag="xr")
            nc.sync.dma_start(out=xr[:], in_=xqr[band])
            nc.vector.tensor_tensor(out=xt[:], in0=xt[:], in1=xr[:],
                                    op=mybir.AluOpType.add)
            stats = small.tile([128, 2, 6], F32, name=f"st_{band}", tag="st")
            for h in range(2):
                nc.vector.bn_stats(out=stats[:, h, :], in_=xt[:, h * 512 : (h + 1) * 512])
            mv = small.tile([128, 2], F32, name=f"mv_{band}", tag="mv")
            nc.vector.bn_aggr(out=mv[:], in_=stats.rearrange("p a b -> p (a b)"))
            rstd = small.tile([128, 1], F32, name=f"rstd_{band}", tag="rstd")
            nc.scalar.activation(
                out=rstd[:],
                in_=mv[:, 1:2],
                func=mybir.ActivationFunctionType.Sqrt,
                bias=eps_sb[:],
            )
            nc.vector.reciprocal(rstd[:], rstd[:])
            nc.vector.tensor_scalar(
                out=xt[:],
                in0=xt[:],
                scalar1=mv[:, 0:1],
                scalar2=rstd[:],
                op0=mybir.AluOpType.subtract,
                op1=mybir.AluOpType.mult,
            )
            nc.vector.tensor_tensor(out=xt[:], in0=xt[:], in1=gamma_bc[:],
                                    op=mybir.AluOpType.mult)
            nc.vector.tensor_tensor(out=xt[:], in0=xt[:], in1=beta_bc[:],
                                    op=mybir.AluOpType.add)
            nc.sync.dma_start(out=out[band], in_=xt[:])

    if iters == 1:
        body(do_tail=tail)
    else:
        load_all()
        with tc.For_i(0, iters):
            body(do_tail=False, lead_load=False, prefetch=True)
        body(do_tail=tail, lead_load=False)

    ctx.close()


def _prepare_inputs(query_seq, key_value_seq, Wq, bq, Wk, bk, Wv, bv, Wo, bo,
                    ln_gamma, ln_beta):
    """Build the 8 per-core input tuples (bf16 compute path)."""
    ins = []
    for c in range(8):
        b, r = divmod(c, 4)
        hs = slice(256 * r, 256 * (r + 1))
        xq = np.ascontiguousarray(query_seq[b]).astype(NP_BF16)
        xkv = np.ascontiguousarray(key_value_seq[b]).astype(NP_BF16)
        wqT = np.ascontiguousarray(Wq[hs, :].T).astype(NP_BF16)
        wkT = np.ascontiguousarray(Wk[hs, :].T).astype(NP_BF16)
        wvT = np.ascontiguousarray(Wv[hs, :].T).astype(NP_BF16)
        woT = np.ascontiguousarray(Wo[:, hs].T).astype(NP_BF16)
        bqk = np.stack(
            [bq[hs][:128], bq[hs][128:], bk[hs][:128], bk[hs][128:]], axis=1
        ).astype(np.float32)
        bvs = np.ascontiguousarray(bv[hs]).astype(np.float32)
        # residual rows (+bo): band k covers rows [512k + 128r, 512k + 128(r+1))
        xqr = np.stack(
            [query_seq[b, 512 * k + 128 * r : 512 * k + 128 * (r + 1)] + bo
             for k in range(4)]
        ).astype(np.float32)
        ins.append((xq, xkv, wqT, wkT, wvT, woT, bqk, bvs,
                    np.ascontiguousarray(ln_gamma).astype(np.float32),
                    np.ascontiguousarray(ln_beta).astype(np.float32), xqr))
    return ins


def kernel(**inputs) -> np.ndarray:
    query_seq = np.asarray(inputs["query_seq"], dtype=np.float32)
    key_value_seq = np.asarray(inputs["key_value_seq"], dtype=np.float32)
    args = {
        k: np.asarray(inputs[k], dtype=np.float32)
        for k in ("Wq", "bq", "Wk", "bk", "Wv", "bv", "Wo", "bo",
                  "ln_gamma", "ln_beta")
    }
    ins = _prepare_inputs(query_seq, key_value_seq, **args)
    out_like = [(np.zeros((4, 128, E), np.float32),) for _ in range(8)]
    res = run_kernel(
        make_attention_kernel(1),
        None,
        ins,
        bass_type=tile.TileContext,
        num_cores=8,
        check_with_sim=False,
        check_with_hw=True,
        output_like=out_like,
    )
    out = np.empty((B, L, E), np.float32)
    for c in range(8):
        bnd = res.results[c]["0_dram"]  # [4, 128, 1024]
        b, r = divmod(c, 4)
        for k in range(4):
            out[b, 512 * k + 128 * r : 512 * k + 128 * (r + 1), :] = bnd[k]
    return out



# revision 2
# speedup vs baseline: 1.5201x; 1.5201x over previous
"""Cross multi-head attention + residual + LayerNorm on 8 Trainium2 NeuronCores.

Reference (per batch b):
    q = x_q @ Wq.T + bq ; k = x_kv @ Wk.T + bk ; v = x_kv @ Wv.T + bv
    per head: ctx = softmax(q k^T / sqrt(64)) v
    out = concat(ctx) @ Wo.T + bo ;  y = LayerNorm(out + x_q) * gamma + beta

Sharding (8 cores): data parallel on batch (2 groups of 4 cores), tensor
parallel on heads (4 of 16 heads per core). Each core computes q/k/v
projections for its 4 heads over the full sequences, attention, and a
partial output projection (its heads' slice of Wo columns); a ReduceScatter
within each 4-core group sums the partials and hands each core 1/4 of the
rows, on which it applies residual + LayerNorm locally (bo is pre-added
into the residual host-side).

Compute path is bf16 (inputs/weights converted host-side), with fp32 PSUM
accumulation, fp32 softmax denominators, and fp32 residual + LayerNorm.
x^T needed by the projections is produced by DMA x-bar transpose loads
(bf16-only HW path) instead of PE transposes.  Softmax skips
max-subtraction (scores ~ N(0,1)) and folds the 1/8 scale into the ACT
exp; the denominator comes from an all-ones column appended to each head's
V block, so the context matmul yields [ctx; denom] in one PSUM pass.

Optimizations over the original bf16 version (285us -> ~210us):
  - inputs arrive pre-transposed host-side ([E, L]); straight 1 MB DMA loads
    instead of x-bar transpose DMAs.
  - fp8e4 (DoubleRow) everywhere on the PE: projections contract e-chunk
    pairs, the context matmul contracts jc-chunk pairs, and the output
    projection contracts both head-chunks in one MM -- about half the PE
    cycles of the bf16 version.  Weights are scaled x16 host-side so they sit
    in fp8's normal range; the 1/16 is folded into the projection evacuation
    ops, and for V into the ones-column value (16.0), which the denominator
    sums, so normalization needs no extra work.
  - exp is emitted as fp8e4 attention weights with a constant -4.2 shift
    (softmax is shift-invariant; keeps exp below fp8e4's max).

Self-contained: hardcodes shapes for B=2, L=2048, E=1024, H=16, Dh=64.
"""

from contextlib import ExitStack

import numpy as np
import ml_dtypes

import concourse.bass as bass
import concourse.mybir as mybir
import concourse.tile as tile
from concourse.bass_test_utils import run_kernel

F32 = mybir.dt.float32
BF16 = mybir.dt.bfloat16
FP8 = mybir.dt.float8e4
NP_BF16 = ml_dtypes.bfloat16
NP_FP8 = ml_dtypes.float8_e4m3

B = 2
L = 2048          # query and kv sequence length
E = 1024          # embed
H_LOC = 4         # heads per core
DH = 64
EC = E // 128     # 8 e-chunks
JC = L // 128     # 16 sequence chunks of 128
IT = 512          # i-tile (moving free dim) for scores/ctx
N_IT = L // IT    # 4
GROUPS = [[0, 1, 2, 3], [4, 5, 6, 7]]
LN_EPS = 1e-5


def make_attention_kernel(iters=1, tail=True):
    def _k(tc, outs, ins):
        return _attention_body(tc, outs, ins, iters, tail)
    return _k


def _attention_body(tc: tile.TileContext, outs, ins, iters, tail=True):
    nc = tc.nc
    (out,) = outs            # [4, 128, 1024] f32 four row-bands of the final output
    (xq, xkv, wqT, wkT, wvT, woT, bqk, bv, gamma, beta, xqr) = ins
    # xq/xkv: [2048, 1024] bf16 (full batch seqs)
    # wqT/wkT/wvT: [1024, 256] bf16 (W.T slice for this core's 4 heads)
    # woT: [256, 1024] bf16 (Wo cols slice, transposed)
    # bqk: [128, 4] f32 (cols: bq pair0, bq pair1, bk pair0, bk pair1)
    # bv: [256] f32 ; gamma/beta: [1024] f32
    # xqr: [4, 128, 1024] f32 residual rows (+bo) matching this core's RS rows

    rs_in = [nc.dram_tensor(f"rs_in{k}", [IT, E], BF16) for k in range(4)]
    rs_out = [nc.dram_tensor(f"rs_out{k}", [128, E], BF16) for k in range(4)]

    ctx = ExitStack()
    singles = ctx.enter_context(tc.tile_pool(name="singles", bufs=1))
    big = ctx.enter_context(tc.tile_pool(name="big", bufs=1))
    ex_pool = ctx.enter_context(tc.tile_pool(name="ex", bufs=6))
    vt_pool = ctx.enter_context(tc.tile_pool(name="vt", bufs=8))
    norm = ctx.enter_context(tc.tile_pool(name="norm", bufs=3))
    small = ctx.enter_context(tc.tile_pool(name="small", bufs=2))
    evac = ctx.enter_context(tc.tile_pool(name="evac", bufs=2))
    ps = ctx.enter_context(tc.tile_pool(name="ps", bufs=2, space="PSUM"))
    psc = ctx.enter_context(tc.tile_pool(name="psc", bufs=2, space="PSUM"))

    # ---- constants / weights -------------------------------------------------
    w_sb = {}
    for name, src, shape in (
        ("wq", wqT, [128, EC, 256]),
        ("wk", wkT, [128, EC, 256]),
        ("wv", wvT, [128, EC, 256]),
        ("wo", woT, [128, 2, E]),
    ):
        wr = singles.tile(shape, FP8, name=f"{name}_r")
        nc.sync.dma_start(out=wr[:], in_=src.rearrange("(c p) n -> p c n", p=128))
        w_sb[name] = wr

    bqk_sb = singles.tile([128, 4], F32, name="bqk_sb")
    nc.sync.dma_start(out=bqk_sb[:], in_=bqk[:])
    bv_bc = singles.tile([128, 256], F32, name="bv_bc")
    nc.gpsimd.dma_start(out=bv_bc[:], in_=bv[None, :].to_broadcast([128, 256]))
    gamma_bc = singles.tile([128, E], F32, name="gamma_bc")
    nc.gpsimd.dma_start(out=gamma_bc[:], in_=gamma[None, :].to_broadcast([128, E]))
    beta_bc = singles.tile([128, E], F32, name="beta_bc")
    nc.gpsimd.dma_start(out=beta_bc[:], in_=beta[None, :].to_broadcast([128, E]))
    eps_sb = singles.tile([128, 1], F32, name="eps_sb")
    nc.vector.memset(eps_sb[:], LN_EPS)
    shift_sb = singles.tile([128, 1], F32, name="shift_sb")
    nc.vector.memset(shift_sb[:], -4.2)
    inv16_sb = singles.tile([128, 1], F32, name="inv16_sb")
    nc.vector.memset(inv16_sb[:], 1.0 / 16.0)


    # V in per-jcp fp8 tiles [128, 2(jc parity), 4*HB] (DoubleRow layout);
    # col 64 of each head block = ones denominator column, cols 65:80 pad.
    HB = 80
    ones_sb = singles.tile([128, 8], F32, name="ones_sb")
    nc.vector.memset(ones_sb[:], 16.0)
    v_tiles = {}

    xkvT = big.tile([128, EC, L], FP8, name="xkvT")   # [e%128, e//128, j]
    xqT = big.tile([128, EC, L], FP8, name="xqT")     # [e%128, e//128, i]
    kT_sb = big.tile([128, 2, L], BF16, name="kT_sb")  # [d(pair), pair, j]
    qT_sb = big.tile([128, 2, L], BF16, name="qT_sb")  # [d(pair), pair, i]
    ctxT_sb = big.tile([128, 2, L], FP8, name="ctxT_sb")  # [hd%128, hd//2? see use]

    def load_T(src, dst, tl):
        nc.sync.dma_start(
            out=dst[:, :, tl * IT : (tl + 1) * IT],
            in_=src.rearrange("(c p) l -> p c l", p=128)[:, :, tl * IT : (tl + 1) * IT],
        )

    def kv_proj(jt):
        for pair in range(2):
            pk = ps.tile([128, IT], F32, name=f"pk_{jt}_{pair}", tag="ps_s")
            for ecp in range(EC // 2):
                nc.tensor.matmul(
                    pk[:],
                    w_sb["wk"][:, 2 * ecp : 2 * ecp + 2,
                               pair * 128 : (pair + 1) * 128],
                    xkvT[:, 2 * ecp : 2 * ecp + 2, jt * IT : (jt + 1) * IT],
                    start=(ecp == 0),
                    stop=(ecp == EC // 2 - 1),
                    perf_mode=mybir.MatmulPerfMode.DoubleRow,
                )
            nc.vector.tensor_scalar(
                out=kT_sb[:, pair, jt * IT : (jt + 1) * IT],
                in0=pk[:],
                scalar1=inv16_sb[:],
                op0=mybir.AluOpType.mult,
                scalar2=bqk_sb[:, 2 + pair : 3 + pair],
                op1=mybir.AluOpType.add,
            )
        for jj in range(4):
            jc = jt * 4 + jj
            pv = ps.tile([128, 256], F32, name=f"pv_{jc}", tag="ps_s")
            for ecp in range(EC // 2):
                nc.tensor.matmul(
                    pv[:],
                    xkvT[:, 2 * ecp : 2 * ecp + 2, jc * 128 : (jc + 1) * 128],
                    w_sb["wv"][:, 2 * ecp : 2 * ecp + 2, :],
                    start=(ecp == 0),
                    stop=(ecp == EC // 2 - 1),
                    perf_mode=mybir.MatmulPerfMode.DoubleRow,
                )
            jcp, par = jc // 2, jc % 2
            if par == 0:
                v_tiles[jcp] = vt_pool.tile([128, 2, H_LOC * HB], FP8,
                                            name=f"v_{jcp}", tag="v")
            vt = v_tiles[jcp]
            with nc.allow_low_precision(reason="attention V in fp8e4"):
                nc.vector.tensor_tensor(
                    out=vt[:, par, :].rearrange("p (h d) -> p h d", d=HB)[:, :, 0:64],
                    in0=pv.rearrange("p (h d) -> p h d", d=64),
                    in1=bv_bc.rearrange("p (h d) -> p h d", d=64),
                    op=mybir.AluOpType.add,
                )
                nc.vector.tensor_copy(
                    vt[:, par, :].rearrange("p (h d) -> p h d", d=HB)[:, :, 64:80],
                    ones_sb[:, 0:4, None].to_broadcast([128, 4, 16]),
                )

    def q_proj(it):
        for pair in range(2):
            pq = ps.tile([128, IT], F32, name=f"pq_{it}_{pair}", tag="ps_s")
            for ecp in range(EC // 2):
                nc.tensor.matmul(
                    pq[:],
                    w_sb["wq"][:, 2 * ecp : 2 * ecp + 2,
                               pair * 128 : (pair + 1) * 128],
                    xqT[:, 2 * ecp : 2 * ecp + 2, it * IT : (it + 1) * IT],
                    start=(ecp == 0),
                    stop=(ecp == EC // 2 - 1),
                    perf_mode=mybir.MatmulPerfMode.DoubleRow,
                )
            nc.vector.tensor_scalar(
                out=qT_sb[:, pair, it * IT : (it + 1) * IT],
                in0=pq[:],
                scalar1=inv16_sb[:],
                op0=mybir.AluOpType.mult,
                scalar2=bqk_sb[:, pair : pair + 1],
                op1=mybir.AluOpType.add,
            )

    ex_cur = {}

    def attn_chunk(pair, it, pc_a, pc_b, jcs):
        ha, hb = 2 * pair, 2 * pair + 1
        for jc in jcs:
            s_ps = ps.tile([128, 2, IT], F32, name=f"sps_{pair}_{it}_{jc}",
                           tag="ps_s2")
            nc.tensor.matmul(
                s_ps[:, 0, :],
                kT_sb[0:64, pair, jc * 128 : (jc + 1) * 128],
                qT_sb[0:64, pair, it * IT : (it + 1) * IT],
                start=True,
                stop=True,
                tile_position=(0, 0),
            )
            nc.tensor.matmul(
                s_ps[:, 1, :],
                kT_sb[64:128, pair, jc * 128 : (jc + 1) * 128],
                qT_sb[64:128, pair, it * IT : (it + 1) * IT],
                start=True,
                stop=True,
                tile_position=(64, 0),
            )
            jcp, par = jc // 2, jc % 2
            if par == 0:
                ex_cur["t"] = ex_pool.tile([128, 2, 2, IT], FP8,
                                           name=f"ex_{pair}_{it}_{jc}", tag="ex")
            ex4 = ex_cur["t"]
            # shift keeps exp inside fp8e4 range (softmax shift-invariant; the
            # ones-column denominator sums the same shifted weights)
            with nc.allow_low_precision(reason="attention weights in fp8e4"):
                nc.scalar.activation(
                    out=ex4[:, par, :, :],
                    in_=s_ps[:],
                    func=mybir.ActivationFunctionType.Exp,
                    scale=0.125,
                    bias=shift_sb[:],
                )
            if par == 1:
                # DoubleRow fp8: each MM contracts both jc chunks of the pair
                nc.tensor.matmul(
                    pc_a[0:65, :],
                    v_tiles[jcp][:, :, ha * HB : ha * HB + 65],
                    ex4[:, :, 0, :],
                    start=(jcp == 0),
                    stop=(jcp == JC // 2 - 1),
                    perf_mode=mybir.MatmulPerfMode.DoubleRow,
                )
                nc.tensor.matmul(
                    pc_b[0:65, :],
                    v_tiles[jcp][:, :, hb * HB : hb * HB + 65],
                    ex4[:, :, 1, :],
                    start=(jcp == 0),
                    stop=(jcp == JC // 2 - 1),
                    perf_mode=mybir.MatmulPerfMode.DoubleRow,
                )

    def attn_norm(pair, it, pc_a, pc_b):
        # Evacuate the PSUM accumulators to SBUF immediately so the psc ring
        # frees for the next slot.  rows 0-63 are ctx^T, row 64 the softmax
        # denominator; reciprocal in place, then a PE matmul with sel64
        # broadcasts the reciprocal row to 64 partitions on-chip (no DRAM
        # round trip), and DVE applies it.
        pcs = norm.tile([128, 2, IT], mybir.dt.float32r,
                        name=f"pcs_{pair}_{it}", tag="pcs")
        nc.vector.tensor_copy(pcs[:, 0, :], pc_a[:])
        nc.vector.tensor_copy(pcs[:, 1, :], pc_b[:])
        with nc.allow_low_precision(reason="f32r has identical fp32 storage"):
            nc.vector.reciprocal(pcs[64:65, 0, :], pcs[64:65, 0, :])
            nc.vector.reciprocal(pcs[64:65, 1, :], pcs[64:65, 1, :])
        # broadcast the reciprocal rows across partitions on the nearly idle
        # GpSimd engine: stage partition 64 -> 0 with a small SBUF->SBUF
        # copy, then partition_broadcast (which reads physical partition 0).
        # No DRAM round trip, no PE/DVE queue involvement.
        stg = norm.tile([1, 2, IT], mybir.dt.float32r,
                        name=f"stg_{pair}_{it}", tag="stg")
        nc.gpsimd.dma_start(out=stg[:], in_=pcs[64:65, :, :])
        bc = norm.tile([64, 2, IT], mybir.dt.float32r,
                       name=f"bc_{pair}_{it}", tag="bc")
        nc.gpsimd.partition_broadcast(bc[:, 0, :], stg[0:1, 0, :], channels=64)
        nc.gpsimd.partition_broadcast(bc[:, 1, :], stg[0:1, 1, :], channels=64)
        return pcs, bc

    def attn_finish(pair, it, pcs, bc):
        # Deferred one slot so the DVE queue never head-of-line blocks on the
        # broadcast chain.
        with nc.allow_low_precision(reason="normalized ctx in fp8e4"):
            nc.vector.tensor_tensor(
                out=ctxT_sb[0:64, pair, it * IT : (it + 1) * IT],
                in0=pcs[0:64, 0, :],
                in1=bc[:, 0, :],
                op=mybir.AluOpType.mult,
            )
            sc = norm.tile([64, IT], FP8, name=f"sc_{pair}_{it}", tag="sc")
            nc.vector.tensor_tensor(
                out=sc[:],
                in0=pcs[0:64, 1, :],
                in1=bc[:, 1, :],
                op=mybir.AluOpType.mult,
            )
        nc.gpsimd.dma_start(
            out=ctxT_sb[64:128, pair, it * IT : (it + 1) * IT],
            in_=sc[:],
        )

    def attn_slot(pair, it):
        pc_a = psc.tile([128, IT], F32, name=f"pca_{pair}_{it}", tag="ps_c")
        pc_b = psc.tile([128, IT], F32, name=f"pcb_{pair}_{it}", tag="ps_c")
        attn_chunk(pair, it, pc_a, pc_b, range(JC))
        return attn_norm(pair, it, pc_a, pc_b)

    def outproj_band(it, do_tail):
        # po tiles use the projection PSUM ring (free during attention) so the
        # scores ring (ps_s2) never stalls on output-projection evacuation.
        # The whole band is staged in one SBUF tile and shipped with a single
        # 1 MB DMA (4 small DMAs would pay 4x the fixed cost).
        ot = evac.tile([128, 4, E], BF16, name=f"ot_{it}", tag="ot")
        for i2 in range(4):
            ic = it * 4 + i2
            for et in range(2):
                po = ps.tile([128, IT], F32, name=f"po_{ic}_{et}", tag="ps_s")
                nc.tensor.matmul(
                    po[:],
                    ctxT_sb[:, :, ic * 128 : (ic + 1) * 128],
                    w_sb["wo"][:, :, et * IT : (et + 1) * IT],
                    start=True,
                    stop=True,
                    perf_mode=mybir.MatmulPerfMode.DoubleRow,
                )
                nc.vector.tensor_scalar(
                    out=ot[:, i2, et * IT : (et + 1) * IT],
                    in0=po[:],
                    scalar1=inv16_sb[:],
                    scalar2=None,
                    op0=mybir.AluOpType.mult,
                )
        nc.sync.dma_start(
            out=rs_in[it].ap().rearrange("(a p) n -> p a n", p=128), in_=ot[:]
        )
        if do_tail:
            nc.gpsimd.collective_compute(
                "ReduceScatter",
                mybir.AluOpType.add,
                replica_groups=GROUPS,
                ins=[rs_in[it].ap()],
                outs=[rs_out[it].ap()],
            )

    def load_all(kv_first=True):
        load_T(xkv, xkvT, 0)
        load_T(xq, xqT, 0)
        for tl in (1, 2, 3):
            load_T(xkv, xkvT, tl)
        for tl in (1, 2, 3):
            load_T(xq, xqT, tl)

    def body(do_tail=True, lead_load=True, prefetch=False):
        # x^T DMA x-bar transpose loads (skipped when the previous loop
        # iteration already prefetched them)
        if lead_load:
            load_all()

        # slot (0,0) starts as soon as kv tile 0 is projected; remaining kv
        # projections interleave with its attention chunks so ACT fills early.
        kv_proj(0)
        q_proj(0)
        pc_a0 = psc.tile([128, IT], F32, name="pca_0_0", tag="ps_c")
        pc_b0 = psc.tile([128, IT], F32, name="pcb_0_0", tag="ps_c")
        attn_chunk(0, 0, pc_a0, pc_b0, range(0, 4))
        kv_proj(1)
        attn_chunk(0, 0, pc_a0, pc_b0, range(4, 8))
        kv_proj(2)
        attn_chunk(0, 0, pc_a0, pc_b0, range(8, 12))
        kv_proj(3)
        attn_chunk(0, 0, pc_a0, pc_b0, range(12, 16))
        prev = (0, 0) + attn_norm(0, 0, pc_a0, pc_b0)

        # Each slot's normalize-finish (and each band's outproj) is emitted
        # one slot late so no engine queue head-of-line blocks on the
        # normalize chain; band it's outproj lands after band it+1 starts.
        for it in range(N_IT):
            for pair in range(2):
                if (pair, it) == (0, 0):
                    continue
                if pair == 0:
                    q_proj(it)
                cur = (pair, it) + attn_slot(pair, it)
                attn_finish(*prev)
                if pair == 1 and it > 0:
                    outproj_band(it - 1, do_tail)
                prev = cur
        if prefetch:
            load_all()
        attn_finish(*prev)
        outproj_band(N_IT - 1, do_tail)

        # ---- residual + LayerNorm per received band -------------------------
        for band in (range(4) if do_tail else []):
            xt = evac.tile([128, E], F32, name=f"xt_{band}", tag="xt")
            nc.gpsimd.dma_start(out=xt[:], in_=rs_out[band].ap())  # bf16 -> f32 cast
            xr = evac.tile([128, E], F32, name=f"xr_{band}", tag="xr")
            nc.sync.dma_start(out=xr[:], in_=xqr[band])
            nc.vector.tensor_tensor(out=xt[:], in0=xt[:], in1=xr[:],
                                    op=mybir.AluOpType.add)
            stats = small.tile([128, 2, 6], F32, name=f"st_{band}", tag="st")
            for h in range(2):
                nc.vector.bn_stats(out=stats[:, h, :], in_=xt[:, h * 512 : (h + 1) * 512])
            mv = small.tile([128, 2], F32, name=f"mv_{band}", tag="mv")
            nc.vector.bn_aggr(out=mv[:], in_=stats.rearrange("p a b -> p (a b)"))
            rstd = small.tile([128, 1], F32, name=f"rstd_{band}", tag="rstd")
            nc.scalar.activation(
                out=rstd[:],
                in_=mv[:, 1:2],
                func=mybir.ActivationFunctionType.Sqrt,
                bias=eps_sb[:],
            )
            nc.vector.reciprocal(rstd[:], rstd[:])
            nc.vector.tensor_scalar(
                out=xt[:],
                in0=xt[:],
                scalar1=mv[:, 0:1],
                scalar2=rstd[:],
                op0=mybir.AluOpType.subtract,
                op1=mybir.AluOpType.mult,
            )
            nc.vector.tensor_tensor(out=xt[:], in0=xt[:], in1=gamma_bc[:],
                                    op=mybir.AluOpType.mult)
            nc.vector.tensor_tensor(out=xt[:], in0=xt[:], in1=beta_bc[:],
                                    op=mybir.AluOpType.add)
            nc.sync.dma_start(out=out[band], in_=xt[:])

    if iters == 1:
        body(do_tail=tail)
    else:
        load_all()
        with tc.For_i(0, iters):
            body(do_tail=False, lead_load=False, prefetch=True)
        body(do_tail=tail, lead_load=False)

    ctx.close()


def _prepare_inputs(query_seq, key_value_seq, Wq, bq, Wk, bk, Wv, bv, Wo, bo,
                    ln_gamma, ln_beta):
    """Build the 8 per-core input tuples (bf16 compute path)."""
    ins = []
    for c in range(8):
        b, r = divmod(c, 4)
        hs = slice(256 * r, 256 * (r + 1))
        xq = np.ascontiguousarray(query_seq[b].T).astype(NP_FP8)
        xkv = np.ascontiguousarray(key_value_seq[b].T).astype(NP_FP8)
        wqT = np.ascontiguousarray(Wq[hs, :].T * 16.0).astype(NP_FP8)
        wkT = np.ascontiguousarray(Wk[hs, :].T * 16.0).astype(NP_FP8)
        wvT = np.ascontiguousarray(Wv[hs, :].T * 16.0).astype(NP_FP8)
        woT = np.ascontiguousarray(Wo[:, hs].T * 16.0).astype(NP_FP8)
        bqk = np.stack(
            [bq[hs][:128], bq[hs][128:], bk[hs][:128], bk[hs][128:]], axis=1
        ).astype(np.float32)
        bvs = np.ascontiguousarray(bv[hs] * 16.0).astype(np.float32)
        # residual rows (+bo): band k covers rows [512k + 128r, 512k + 128(r+1))
        xqr = np.stack(
            [query_seq[b, 512 * k + 128 * r : 512 * k + 128 * (r + 1)] + bo
             for k in range(4)]
        ).astype(np.float32)
        ins.append((xq, xkv, wqT, wkT, wvT, woT, bqk, bvs,
                    np.ascontiguousarray(ln_gamma).astype(np.float32),
                    np.ascontiguousarray(ln_beta).astype(np.float32), xqr))
    return ins


def kernel(**inputs) -> np.ndarray:
    query_seq = np.asarray(inputs["query_seq"], dtype=np.float32)
    key_value_seq = np.asarray(inputs["key_value_seq"], dtype=np.float32)
    args = {
        k: np.asarray(inputs[k], dtype=np.float32)
        for k in ("Wq", "bq", "Wk", "bk", "Wv", "bv", "Wo", "bo",
                  "ln_gamma", "ln_beta")
    }
    ins = _prepare_inputs(query_seq, key_value_seq, **args)
    out_like = [(np.zeros((4, 128, E), np.float32),) for _ in range(8)]
    res = run_kernel(
        make_attention_kernel(1),
        None,
        ins,
        bass_type=tile.TileContext,
        num_cores=8,
        check_with_sim=False,
        check_with_hw=True,
        output_like=out_like,
    )
    out = np.empty((B, L, E), np.float32)
    for c in range(8):
        bnd = res.results[c]["0_dram"]  # [4, 128, 1024]
        b, r = divmod(c, 4)
        for k in range(4):
            out[b, 512 * k + 128 * r : 512 * k + 128 * (r + 1), :] = bnd[k]
    return out

